# revision 33
# baseline (speedup 1.0000x reference)
"""Trainium2 Bass kernel for nn_Attn_71322226917754.

Additive (Bahdanau-style) attention with length masking:
  energy[b,d,e] = v . tanh(We@enc[b,e] + Wd@dec[b,d] + W_b)   (+v_b, cancels in softmax)
  attn = masked softmax over e;  context[b,d] = sum_e attn * enc[b,e]

Strategy: only rows (b, d<dec_len[b]) contribute (others are zero), and only
e < enc_len[b] columns matter.  The host packs all valid rows, balances them
across 8 NeuronCores (work ~ enc_len per row), groups each core's rows into
per-batch "segments", and pads to an SPMD-uniform segment structure (max
rows/extent per slot across cores).  Per-core data differs; program is shared.

Device per segment (batch b, N rows, extent EXT):
  peT[k,e]  = WeT.T @ encT          (PE; encT host-pretransposed)
  pdb[k,r]  = WdT.T @ decT + W_b    (PE + DVE, once for all rows)
  pre       = peT + pdb[:,r]        (DVE tensor_scalar, 2x fp32)
  tanh      = ACT over G-row groups
  energy[r] = v.T @ tanh            (PE, M=1 matmuls into shared PSUM bank)
  energy   += negmask (-1e30 beyond enc_len; energies are tanh-bounded so no
              max-subtraction is needed; exp(-1e30)=0 exactly)
  exp + row-sum (ACT accum_out), reciprocal, attn = exp * 1/s (DVE)
  attnT (PE transpose) ; context = attnT.T @ enc (PE, accumulate over e-chunks)
Host scatters returned rows into the (16,64,128) output.
"""

import os
import numpy as np

B, E, D, H = 16, 512, 64, 128
NCORES = 8
NEG = -1.0e30
G = 8   # rows per tanh group

LAST_RESULT = None  # BassKernelResults from the most recent run (for test.py)
LAST_NC = None      # the built Bass program (for test.py timeline analysis)


# ----------------------------------------------------------------- packing
def _build_slots(order, el, dl, maxn, thr):
    """Stream batches (el desc) into slots of 8 cells, one batch per cell,
    splitting batches across cells and spreading each batch's rows evenly
    over its cells.  Close a slot when cells run out or el drops below
    thr * slot extent.  Returns (slots, core_segs) with
    core_segs[c][j] = (b, d_list, el_b) or (-1, [], 0)."""
    slots, core_cells = [], []
    queue = [(b, list(range(int(dl[b])))) for b in order]
    qi = 0
    while qi < len(queue):
        ext = int(el[queue[qi][0]])
        taken = []  # [b, rows, ncells]
        used = 0
        while qi < len(queue) and used < NCORES:
            b, ds = queue[qi]
            if taken and int(el[b]) < thr * ext:
                break
            ncell = min((len(ds) + maxn - 1) // maxn, NCORES - used)
            rows = ds[:ncell * maxn]
            taken.append([b, rows, ncell])
            queue[qi] = (b, ds[len(rows):])
            if not queue[qi][1]:
                qi += 1
            used += ncell
        # hand spare cells to whoever has the tallest cells
        spare = NCORES - used
        while spare > 0:
            cand = max(taken, key=lambda t: -(-len(t[1]) // t[2]))
            if -(-len(cand[1]) // cand[2]) <= -(-len(cand[1]) // (cand[2] + 1)):
                break
            cand[2] += 1
            spare -= 1
        cells = []
        nmax = 0
        for b, rows, ncell in taken:
            q, r = divmod(len(rows), ncell)
            o = 0
            for i in range(ncell):
                take = q + (1 if i < r else 0)
                cells.append((b, rows[o:o + take]))
                o += take
                nmax = max(nmax, take)
        cells += [None] * (NCORES - len(cells))
        slots.append((nmax, max(4, min(E, 4 * ((ext + 3) // 4)))))
        core_cells.append(cells)
    core_segs = [[] for _ in range(NCORES)]
    for j, cells in enumerate(core_cells):
        for c in range(NCORES):
            if cells[c] is not None and cells[c][1]:
                b, ds = cells[c]
                core_segs[c].append((b, ds, int(el[b])))
            else:
                core_segs[c].append((-1, [], 0))
    return slots, core_segs


def _pack(el, dl):
    """Returns (slots, core_segs): slots = [(N_j, EXT_j)] uniform across
    cores; core_segs[c][j] = (b, d_list, el_b), b == -1 for dummy."""
    order = sorted((b for b in range(B) if el[b] > 0 and dl[b] > 0),
                   key=lambda b: -el[b])
    total_rows = sum(int(dl[b]) for b in order)
    best, best_score = None, None
    for maxn in (8, 10, 12, 16, 20, 24, 32, 48, 64):
        for thr in (0.0, 0.5, 0.65, 0.8, 0.9, 0.95):
            slots, core_segs = _build_slots(order, el, dl, maxn, thr)
            if max(n for n, _ in slots) > 128:
                continue
            placed = sum(len(s[1]) for cs in core_segs for s in cs)
            assert placed == total_rows, (placed, total_rows)
            act = pe = dve = 0.0
            for n, e in slots:
                act += (n * e + e + 352 * (2 + (n + G - 1) // G)) / 1.2
                pe += (n * e * (1 if e >= 256 else 4) + 4 * 129 *
                       ((e + 127) // 128) + e) / 2.4
                dve += (n * (e / 2 + 58) + 2.2 * e + 800) / 0.96
            score = max(act, pe, dve) + 900 * len(slots)
            if best_score is None or score < best_score:
                best, best_score = (slots, core_segs), score
    assert best is not None
    slots, core_segs = best
    # smallest slot first: fills the pipeline fast, and the (second-smallest)
    # final slot keeps the serial drain tail short
    smallest = min(range(len(slots)), key=lambda j: slots[j][0] * slots[j][1])
    order_j = [smallest] + [j for j in range(len(slots)) if j != smallest]
    slots = [slots[j] for j in order_j]
    core_segs = [[cs[j] for j in order_j] for cs in core_segs]
    return slots, core_segs


# ----------------------------------------------------------------- program
def _build_program(slots):
    import concourse.bacc as bacc
    import concourse.mybir as mybir
    from concourse.tile import TileContext
    from contextlib import ExitStack

    f32 = mybir.dt.float32
    AF = mybir.ActivationFunctionType
    # fp32 matmuls stream at 4 cycles/row on PE; float32r streams at 1
    # cycle/row once the moving dim is >=256, so type the energy/peT matmul
    # operands float32r end-to-end (the BIR verifier requires producers of
    # fp32r-matmul inputs to emit fp32r).  ctx/pdb stay plain fp32 (no speed
    # difference at their sizes, and full precision there is free).
    use_f32r = bool(int(os.environ.get("BASS_KERNEL_F32R", "1")))
    f32r = mybir.dt.float32r if use_f32r else f32
    NR = sum(n for n, _ in slots)
    EXTMAX = max(e for _, e in slots)
    NCHMAX = (EXTMAX + 127) // 128

    nc = bacc.Bacc("TRN2", target_bir_lowering=False, debug=False,
                   num_devices=NCORES)

    # enc{j}: [e, H+1] — columns 0..H-1 are enc zeroed beyond enc_len, column
    # H is the 0/1 validity mask.  The context matmul then yields both the
    # context rows and the masked softmax denominator in its last column.
    enc_d, encT_d = [], []
    for j, (nj, ej) in enumerate(slots):
        nch = (ej + 127) // 128
        enc_d.append(nc.dram_tensor(f"enc{j}", [nch * 128, H + 1], f32,
                                    kind="ExternalInput").ap())
        encT_d.append(nc.dram_tensor(f"encT{j}", [H, nch * 128], f32,
                                     kind="ExternalInput").ap())
    decT_d = nc.dram_tensor("decT", [H, NR], f32, kind="ExternalInput").ap()
    wet_d = nc.dram_tensor("WeT", [H, H], f32, kind="ExternalInput").ap()
    wdt_d = nc.dram_tensor("WdT", [H, H], f32, kind="ExternalInput").ap()
    # 32 column-shifted copies of v: block g has v in column g, zeros else.
    # lhsT=vshift[:, 32g:32g+32] makes a [32, EXT] matmul deposit row g only;
    # accumulating 32 of these builds a 32-row energy block at a legal PE
    # quadrant origin (matmul out base partition must be 0/32/64/96).
    vs_d = nc.dram_tensor("vshift", [H, 32 * 32], f32r,
                          kind="ExternalInput").ap()
    wb_d = nc.dram_tensor("Wb_col", [H, 1], f32, kind="ExternalInput").ap()
    id_d = nc.dram_tensor("ident", [128, 128], f32, kind="ExternalInput").ap()
    out_d = nc.dram_tensor("out_rows", [NR, H], f32,
                           kind="ExternalOutput").ap()

    with TileContext(nc) as tc, ExitStack() as ctx:
        const = ctx.enter_context(tc.tile_pool(name="const", bufs=1))
        enc_pool = ctx.enter_context(tc.tile_pool(name="encp", bufs=4))
        encT_pool = ctx.enter_context(tc.tile_pool(name="encTp", bufs=4))
        petsb_pool = ctx.enter_context(tc.tile_pool(name="petsb", bufs=3))
        pre_pool = ctx.enter_context(tc.tile_pool(name="prep", bufs=3))
        tanh_pool = ctx.enter_context(tc.tile_pool(name="tanhp", bufs=3))
        exp_pool = ctx.enter_context(tc.tile_pool(name="expp", bufs=3))
        attnT_pool = ctx.enter_context(tc.tile_pool(name="attnTp", bufs=3))
        small_pool = ctx.enter_context(tc.tile_pool(name="smallp", bufs=4))
        ctxsb_pool = ctx.enter_context(tc.tile_pool(name="ctxsb", bufs=2))
        pet_ps_pool = ctx.enter_context(
            tc.tile_pool(name="petps", bufs=2, space="PSUM"))
        energy_pool = ctx.enter_context(
            tc.tile_pool(name="energyps", bufs=3, space="PSUM"))
        tp_pool = ctx.enter_context(
            tc.tile_pool(name="tpps", bufs=1, space="PSUM"))
        ctxps_pool = ctx.enter_context(
            tc.tile_pool(name="ctxps", bufs=2, space="PSUM"))

        HX = H + 1
        def slot_dmas(j, NCH):
            enc_sb = enc_pool.tile([128, NCHMAX * HX], f32, tag="enc")
            enc_src = enc_d[j].rearrange("(ch p) hx -> p ch hx", p=128)
            nc.sync.dma_start(
                enc_sb[:, :NCH * HX].rearrange("p (ch hx) -> p ch hx", hx=HX),
                enc_src)
            encT_sb = encT_pool.tile([128, NCHMAX * 128], f32, tag="encT")
            nc.sync.dma_start(encT_sb[:, :NCH * 128], encT_d[j][:])
            return enc_sb, encT_sb

        premade = {0: slot_dmas(0, (slots[0][1] + 127) // 128)}

        wet_sb = const.tile([H, H], f32, tag="wet")
        nc.sync.dma_start(wet_sb[:], wet_d[:])
        wdt_sb = const.tile([H, H], f32, tag="wdt")
        nc.sync.dma_start(wdt_sb[:], wdt_d[:])
        vs_sb = const.tile([H, 32 * 32], f32r, tag="vshift")
        nc.sync.dma_start(vs_sb[:], vs_d[:])
        wb_sb = const.tile([H, 1], f32, tag="wb")
        nc.sync.dma_start(wb_sb[:], wb_d[:])
        decT_sb = const.tile([H, NR], f32, tag="decT")
        nc.sync.dma_start(decT_sb[:, :NR], decT_d[:])

        # touch Tanh right away so the ~2.7us ACT table load (exp_and_others,
        # which also covers Exp) overlaps the input DMAs
        warm_sb = const.tile([1, 4], f32, tag="warm")
        nc.gpsimd.memset(warm_sb[:, :], 0.0)
        nc.scalar.activation(warm_sb[:1, :], warm_sb[:1, :], AF.Tanh)
        id_sb = const.tile([128, 128], f32, tag="ident")
        nc.sync.dma_start(id_sb[:], id_d[:])

        # pdb[k, r] = Wd @ dec_row + W_b, all packed rows (512/bank chunks)
        pdb_sb = const.tile([H, NR], f32, tag="pdb")
        for o in range(0, NR, 512):
            w = min(512, NR - o)
            pdb_ps = pet_ps_pool.tile([128, 512], f32, tag="pet")
            nc.tensor.matmul(pdb_ps[:, :w], lhsT=wdt_sb[:],
                             rhs=decT_sb[:, o:o + w], start=True, stop=True)
            nc.vector.tensor_scalar_add(pdb_sb[:, o:o + w], pdb_ps[:, :w],
                                        wb_sb[:, 0:1])

        r0 = 0
        for j, (N, EXT) in enumerate(slots):
            NCH = (EXT + 127) // 128
            if j in premade:
                enc_sb, encT_sb = premade[j]
            else:
                enc_sb, encT_sb = slot_dmas(j, NCH)
            if j + 1 < len(slots):
                premade[j + 1] = slot_dmas(j + 1,
                                           (slots[j + 1][1] + 127) // 128)

            pet_ps = pet_ps_pool.tile([128, 512], f32, tag="pet")
            nc.tensor.matmul(pet_ps[:, :EXT], lhsT=wet_sb[:],
                             rhs=encT_sb[:, :EXT], start=True, stop=True)
            pet_sb = petsb_pool.tile([128, 512], f32, tag="pet_sb")
            nc.vector.tensor_copy(pet_sb[:, :EXT], pet_ps[:, :EXT])

            energy_ps = energy_pool.tile([128, 512], f32, tag="energy")
            for g0 in range(0, N, G):
                gn = min(G, N - g0)
                pre = pre_pool.tile([128, G * EXTMAX], f32, tag="pre")
                for i in range(gn):
                    nc.vector.tensor_scalar_add(
                        pre[:, i * EXT:(i + 1) * EXT], pet_sb[:, :EXT],
                        pdb_sb[:, r0 + g0 + i:r0 + g0 + i + 1])
                th = tanh_pool.tile([128, G * EXTMAX], f32r, tag="tanh")
                nc.scalar.activation(th[:, :gn * EXT], pre[:, :gn * EXT],
                                     AF.Tanh)
                for i in range(gn):
                    r = g0 + i          # row within segment
                    q, g = (r // 32) * 32, r % 32
                    nc.tensor.matmul(
                        energy_ps[q:q + 32, :EXT],
                        lhsT=vs_sb[:, g * 32:(g + 1) * 32],
                        rhs=th[:, i * EXT:(i + 1) * EXT],
                        start=(g == 0),
                        stop=(g == 31 or r == N - 1))

            exp_sb = exp_pool.tile([128, E], f32, tag="exp")
            nc.scalar.activation(exp_sb[:N, :EXT], energy_ps[:N, :EXT],
                                 AF.Exp)

            expT_sb = attnT_pool.tile([128, NCHMAX * 128], f32, tag="attnT")
            for ch in range(NCH):
                chw = min(128, EXT - ch * 128)
                tp = tp_pool.tile([128, 128], f32, tag="tp")
                nc.tensor.transpose(tp[:chw, :N],
                                    exp_sb[:N, ch * 128:ch * 128 + chw],
                                    id_sb[:N, :N])
                nc.vector.tensor_copy(expT_sb[:chw, ch * 128:ch * 128 + N],
                                      tp[:chw, :N])
            # ctx_ps[:, :H] = sum_e exp * enc ; ctx_ps[:, H] = sum_e exp*mask
            ctx_ps = ctxps_pool.tile([128, HX], f32, tag="ctx")
            for ch in range(NCH):
                chw = min(128, EXT - ch * 128)
                nc.tensor.matmul(
                    ctx_ps[:N, :HX],
                    lhsT=expT_sb[:chw, ch * 128:ch * 128 + N],
                    rhs=enc_sb[:chw, ch * HX:(ch + 1) * HX],
                    start=(ch == 0), stop=(ch == NCH - 1))
            rec_sb = small_pool.tile([128, 1], f32, tag="rec")
            nc.vector.reciprocal(rec_sb[:N, 0:1], ctx_ps[:N, H:HX])
            ctx_sb = ctxsb_pool.tile([128, H], f32, tag="ctxsb")
            nc.vector.tensor_scalar_mul(ctx_sb[:N, :], ctx_ps[:N, :H],
                                        rec_sb[:N, 0:1])
            nc.sync.dma_start(out_d[r0:r0 + N, :], ctx_sb[:N, :])
            r0 += N

    nc.finalize()  # Bacc register allocation etc.; required before compile
    return nc


# ------------------------------------------------------------------ driver
def kernel(encoder_outputs, decoder_outputs, W_w, W_b, v_w, v_b,
           encoder_length, decoder_length):
    global LAST_RESULT
    from concourse.bass_utils import run_bass_kernel_spmd

    enc = np.ascontiguousarray(np.asarray(encoder_outputs, dtype=np.float32))
    dec = np.ascontiguousarray(np.asarray(decoder_outputs, dtype=np.float32))
    W_w = np.asarray(W_w, dtype=np.float32)
    W_b = np.asarray(W_b, dtype=np.float32)
    v_w = np.asarray(v_w, dtype=np.float32)
    el = np.asarray(encoder_length).astype(np.int64)
    dl = np.asarray(decoder_length).astype(np.int64)

    slots, core_segs = _pack(el, dl)
    NR = sum(n for n, _ in slots)

    wet = np.ascontiguousarray(W_w[:, :H].T)      # [h, k]
    wdt = np.ascontiguousarray(W_w[:, H:].T)      # [h, k]
    vshift = np.zeros((H, 32 * 32), np.float32)
    for g in range(32):
        vshift[:, g * 32 + g] = v_w[0]
    wb_col = np.ascontiguousarray(W_b.reshape(H, 1))
    ident = np.eye(128, dtype=np.float32)

    in_maps = []
    scatter = []  # (core, row, b, d)
    for c in range(NCORES):
        m = {"WeT": wet, "WdT": wdt, "vshift": vshift, "Wb_col": wb_col,
             "ident": ident}
        decT = np.zeros((H, NR), np.float32)
        r0 = 0
        for j, (N, EXT) in enumerate(slots):
            nch = (EXT + 127) // 128
            b, ds, elb = core_segs[c][j]
            e_sl = np.zeros((nch * 128, H + 1), np.float32)
            if b >= 0:
                n = len(ds)
                ncopy = min(nch * 128, min(elb, E))
                e_sl[:ncopy, :H] = enc[b, :ncopy]
                e_sl[:ncopy, H] = 1.0
                decT[:, r0:r0 + n] = dec[b, ds].T
                for i, d in enumerate(ds):
                    scatter.append((c, r0 + i, b, d))
            else:
                e_sl[0, H] = 1.0  # keep s > 0 on dummy cells (no inf/NaN)
            m[f"enc{j}"] = np.ascontiguousarray(e_sl)
            m[f"encT{j}"] = np.ascontiguousarray(e_sl[:, :H].T)
            r0 += N
        m["decT"] = decT
        in_maps.append(m)

    global LAST_NC
    nc = _build_program(slots)
    LAST_NC = nc
    trace = bool(int(os.environ.get("BASS_KERNEL_TRACE", "0")))
    res = run_bass_kernel_spmd(nc, in_maps, core_ids=list(range(NCORES)),
                               trace=trace)
    LAST_RESULT = res

    out = np.zeros((B, D, H), np.float32)
    if scatter:
        sc = np.array(scatter, np.int64)
        rows = np.stack([res.results[c]["out_rows"][r]
                         for c, r in zip(sc[:, 0], sc[:, 1])])
        out[sc[:, 2], sc[:, 3]] = rows
    return out
